# revision 1
# baseline (speedup 1.0000x reference)
"""Causal self-attention Bass/Tile kernel for Trainium2, SPMD over 8 NeuronCores.

Sharding: tensor-parallel over heads. Core c owns heads [2c, 2c+1] (a
128-wide slice of the 1024-dim hidden). Each core:
  stage 1: q/k/v projections for its head slice (PSUM-accumulated over
           the 1024 contraction dim),
  stage 2: causal flash-style attention for its 2 heads x 2 batches
           (scores computed transposed [j, i] with causal-shrunk i-windows;
           exp on ScalarE; softmax denominator via an appended ones-column
           in the V matmul; no max-subtraction -- scores are bounded for
           this problem),
  stage 3: partial output projection out_c = h_c @ Wo[:, slice].T.
Host sums the 8 partial outputs.

Matmul operands use float32r (single-pass fp32 streaming on the PE,
4x the rate of plain fp32; ~2e-4 relative rounding vs fp32). Built with
bacc.Bacc + nc.compile() so multi-semaphore waits are legalized via event
semaphores (this walrus rejects >1 sync wait per raw instruction).
"""

import os
import sys

sys.path.insert(0, "/opt/trn_rl_repo")
os.environ.setdefault("MYCRO_LOCAL_CACHE", "1")

from contextlib import ExitStack

import numpy as np

import concourse.bass as bass
import concourse.tile as tile
from concourse import bacc, mybir

F32 = mybir.dt.float32
F32R = mybir.dt.float32r

B, S, D = 2, 2048, 1024
H, HD = 16, 64
NCORES = 8
HS = 128          # head-slice width per core (2 heads x 64)
KC = D // 128     # contraction chunks for projections

# module-level knobs for test harness
PROFILE = False
LAST_EXEC_NS = None
LAST_RESULTS = None

_PROGRAM_CACHE = {}


def _emit(tc, out, xT, wq, wk, wv, wo, tri, ident, b, s):
    nc = tc.nc
    m = b * s
    n_mt1 = m // 512      # stage-1 m-tiles (moving dim)
    n_it = s // 512       # i-tiles per (batch) unit
    n_jb = s // 128       # j-blocks per unit

    ctx = ExitStack()
    with ctx:
        consts = ctx.enter_context(tc.tile_pool(name="consts", bufs=1))
        persist = ctx.enter_context(tc.tile_pool(name="persist", bufs=1))

        wq_sb = consts.tile([128, KC, 128], F32R)
        wk_sb = consts.tile([128, KC, 128], F32R)
        wv_sb = consts.tile([128, KC, 128], F32R)
        wo_sb = consts.tile([HS, D], F32R)
        tri_sb = consts.tile([128, 128], F32R)
        id_sb = consts.tile([128, 128], F32)
        ones_sb = consts.tile([1, 64], F32R)
        dma = nc.default_dma_engine
        # only wq chunk 0 gates the very first matmul; land it first and
        # defer everything else behind the first x-tile DMAs
        dma.dma_start(out=wq_sb[:, 0:1, :], in_=wq[:, 0:1, :])
        dma.dma_start(out=wq_sb[:, 1:KC, :], in_=wq[:, 1:KC, :])
        late_consts = [
            (wk_sb, wk),
            (wv_sb, wv),
            (wo_sb, wo),
            (id_sb, ident),
        ]
        nc.gpsimd.dma_start(out=tri_sb, in_=tri)  # f32->f32r cast; gpsimd-only
        nc.vector.memset(ones_sb.bitcast(F32), 1.0)

        qT = persist.tile([128, m], F32R)  # rows 0-63 head A dims, 64-127 head B
        kT = persist.tile([128, m], F32R)
        vT = persist.tile([128, m], F32)
        hT = persist.tile([128, m], F32R)  # normalized attention out (transposed)
        vext = persist.tile([128, b * n_jb * 2, 65], F32R)  # v[j,d] + ones col

        # ---------------- stage 1: q/k/v projections ----------------
        with tc.tile_pool(name="xt", bufs=12) as xt_pool, tc.tile_pool(
            name="ps1", bufs=6, space=bass.MemorySpace.PSUM
        ) as ps1:
            for mt in range(n_mt1):
                xts = []
                for kc_i in range(KC):
                    t = xt_pool.tile([128, 512], F32R, tag="xt")
                    dma.dma_start(
                        out=t,
                        in_=xT[
                            kc_i * 128 : (kc_i + 1) * 128, mt * 512 : (mt + 1) * 512
                        ],
                    )
                    xts.append(t)
                if mt == 0:
                    for dst_t, src_ap in late_consts:
                        dma.dma_start(out=dst_t, in_=src_ap)
                for pi, (wsb, dest) in enumerate(
                    [(wq_sb, qT), (wk_sb, kT), (wv_sb, vT)]
                ):
                    acc = ps1.tile([128, 512], F32, tag="acc")
                    for kc_i in range(KC):
                        nc.tensor.matmul(
                            acc,
                            lhsT=wsb[:, kc_i, :],
                            rhs=xts[kc_i],
                            start=(kc_i == 0),
                            stop=(kc_i == KC - 1),
                        )
                    dst = dest[:, mt * 512 : (mt + 1) * 512]
                    if pi == 1:
                        nc.scalar.copy(out=dst, in_=acc)
                    else:
                        nc.vector.tensor_copy(dst, acc)

        # ---------------- v transposes: vT -> v[j, d] chunks ----------------
        with tc.tile_pool(name="pst", bufs=2, space=bass.MemorySpace.PSUM) as pst:
            nc.vector.memset(vext[:, :, 64:65].bitcast(F32), 1.0)
            for u in range(b):
                for jb in range(n_jb):
                    tp = pst.tile([128, 128], F32, tag="tp")
                    col = u * s + jb * 128
                    nc.tensor.transpose(tp, vT[:, col : col + 128], id_sb)
                    idx = (u * n_jb + jb) * 2
                    nc.vector.tensor_copy(vext[:, idx, 0:64], tp[:, 0:64])
                    nc.vector.tensor_copy(vext[:, idx + 1, 0:64], tp[:, 64:128])

        # ---------------- stage 2: causal attention ----------------
        with tc.tile_pool(
            name="ps2s", bufs=2, space=bass.MemorySpace.PSUM
        ) as sp_pool, tc.tile_pool(
            name="ps2o", bufs=1, space=bass.MemorySpace.PSUM
        ) as ops_pool, tc.tile_pool(
            name="ps2b", bufs=1, space=bass.MemorySpace.PSUM
        ) as bc_pool, tc.tile_pool(name="pab", bufs=4) as pab_pool, tc.tile_pool(
            name="rp", bufs=3
        ) as rp_pool, tc.tile_pool(
            name="ps3i", bufs=1, space=bass.MemorySpace.PSUM
        ) as ps3i, tc.tile_pool(name="osb", bufs=6) as osb_pool:
            escale = 1.0 / np.sqrt(HD)
            for u in range(b):
                ucol = u * s
                for it in range(n_it):
                    oA = ops_pool.tile([65, 512], F32, tag="oA")
                    oB = ops_pool.tile([65, 512], F32, tag="oB")
                    jb_hi = 4 * it + 4
                    icol = ucol + it * 512
                    for jb in range(jb_hi):
                        jcol = ucol + jb * 128
                        # causal: this j-block only reaches queries i >= jb*128
                        off = max(0, jb * 128 - it * 512)
                        sab = sp_pool.tile([128, 1024], F32, tag="sab")
                        nc.tensor.matmul(
                            sab[:, off:512],
                            lhsT=kT[0:64, jcol : jcol + 128],
                            rhs=qT[0:64, icol + off : icol + 512],
                            start=True,
                            stop=True,
                        )
                        nc.tensor.matmul(
                            sab[:, 512 + off : 1024],
                            lhsT=kT[64:128, jcol : jcol + 128],
                            rhs=qT[64:128, icol + off : icol + 512],
                            start=True,
                            stop=True,
                        )
                        pab = pab_pool.tile([128, 1024], F32R, tag="pab")
                        if off == 0:
                            nc.scalar.activation(
                                pab,
                                sab,
                                mybir.ActivationFunctionType.Exp,
                                scale=escale,
                            )
                        else:
                            nc.scalar.activation(
                                pab[:, off:512],
                                sab[:, off:512],
                                mybir.ActivationFunctionType.Exp,
                                scale=escale,
                            )
                            nc.scalar.activation(
                                pab[:, 512 + off : 1024],
                                sab[:, 512 + off : 1024],
                                mybir.ActivationFunctionType.Exp,
                                scale=escale,
                            )
                        if jb >= 4 * it:  # diagonal block: causal mask
                            nc.gpsimd.tensor_mul(
                                pab[:, off : off + 128],
                                pab[:, off : off + 128],
                                tri_sb,
                            )
                            nc.gpsimd.tensor_mul(
                                pab[:, 512 + off : 512 + off + 128],
                                pab[:, 512 + off : 512 + off + 128],
                                tri_sb,
                            )
                        idx = (u * n_jb + jb) * 2
                        last = jb == jb_hi - 1
                        nc.tensor.matmul(
                            oA[:, off:512],
                            lhsT=vext[:, idx, :],
                            rhs=pab[:, off:512],
                            start=(jb == 0),
                            stop=last,
                        )
                        nc.tensor.matmul(
                            oB[:, off:512],
                            lhsT=vext[:, idx + 1, :],
                            rhs=pab[:, 512 + off : 1024],
                            start=(jb == 0),
                            stop=last,
                        )
                    ocol = slice(ucol + it * 512, ucol + (it + 1) * 512)
                    nc.vector.tensor_copy(hT[0:64, ocol], oA[0:64, :])
                    nc.vector.tensor_copy(hT[64:128, ocol], oB[0:64, :])
                    for h_i, o_ps in ((0, oA), (1, oB)):
                        # custom-DVE reciprocal misreads PSUM sources on HW;
                        # stage the den row through SBUF first
                        den_sb = rp_pool.tile([1, 512], F32, tag="den")
                        recip1 = rp_pool.tile([1, 512], F32, tag="rc")
                        recip_r = rp_pool.tile([1, 512], F32R, tag="rr")
                        nc.vector.tensor_copy(den_sb[0:1, :], o_ps[64:65, :])
                        nc.vector.reciprocal_approx_fast(
                            out=recip1[0:1, :], in_=den_sb[0:1, :]
                        )
                        nc.vector.tensor_copy(recip_r[0:1, :], recip1[0:1, :])
                        bc = bc_pool.tile([64, 512], F32, tag="bc")
                        nc.tensor.matmul(
                            bc,
                            lhsT=ones_sb,
                            rhs=recip_r[0:1, :],
                            start=True,
                            stop=True,
                        )
                        seg = slice(h_i * 64, (h_i + 1) * 64)
                        nc.vector.tensor_mul(hT[seg, ocol], hT[seg, ocol], bc)
                    # ---- fused stage 3: this i-window's 4 output tiles ----
                    # (emitted inline so the copies and out-DMAs hide under
                    # the next i-tile's attention matmuls)
                    for q4 in range(4):
                        mt3 = (ucol + it * 512) // 128 + q4
                        lhs = hT[:, mt3 * 128 : (mt3 + 1) * 128]
                        ot = osb_pool.tile([128, D], F32, tag="ot")
                        for half in range(2):
                            op = ps3i.tile([128, 512], F32, tag="op")
                            cs = slice(half * 512, (half + 1) * 512)
                            nc.tensor.matmul(
                                op,
                                lhsT=lhs,
                                rhs=wo_sb[:, cs],
                                start=True,
                                stop=True,
                            )
                            if half == 0:
                                nc.vector.tensor_copy(ot[:, cs], op)
                            else:
                                nc.scalar.copy(out=ot[:, cs], in_=op)
                        dma.dma_start(
                            out=out[mt3 * 128 : (mt3 + 1) * 128, :], in_=ot
                        )


def _declare_io(nc, m):
    xT = nc.dram_tensor("xT", [D, m], F32R, kind="ExternalInput").ap()
    wq = nc.dram_tensor("wq", [128, KC, 128], F32R, kind="ExternalInput").ap()
    wk = nc.dram_tensor("wk", [128, KC, 128], F32R, kind="ExternalInput").ap()
    wv = nc.dram_tensor("wv", [128, KC, 128], F32R, kind="ExternalInput").ap()
    wo = nc.dram_tensor("wo", [HS, D], F32R, kind="ExternalInput").ap()
    out = nc.dram_tensor("out", [m, D], F32, kind="ExternalOutput").ap()
    tri = nc.inline_tensor(
        np.triu(np.ones((128, 128), dtype=np.float32)), "tri"
    ).ap()
    ident = nc.inline_tensor(np.eye(128, dtype=np.float32), "ident").ap()
    return xT, wq, wk, wv, wo, out, tri, ident


def build_program(b=B, s=S):
    key = (b, s)
    if key in _PROGRAM_CACHE:
        return _PROGRAM_CACHE[key]
    nc = bacc.Bacc("TRN2", target_bir_lowering=False, debug=False, num_devices=NCORES)
    xT, wq, wk, wv, wo, out, tri, ident = _declare_io(nc, b * s)
    with tile.TileContext(nc) as tc:
        _emit(tc, out, xT, wq, wk, wv, wo, tri, ident, b, s)
    nc.compile()
    _PROGRAM_CACHE[key] = nc
    return nc


def build_program_repeated(b, s, reps):
    """Same program with the pipeline emitted `reps` times (slope timing)."""
    key = (b, s, "rep", reps)
    if key in _PROGRAM_CACHE:
        return _PROGRAM_CACHE[key]
    nc = bacc.Bacc("TRN2", target_bir_lowering=False, debug=False, num_devices=NCORES)
    xT, wq, wk, wv, wo, out, tri, ident = _declare_io(nc, b * s)
    with tile.TileContext(nc) as tc:
        for _ in range(reps):
            _emit(tc, out, xT, wq, wk, wv, wo, tri, ident, b, s)
    nc.compile()
    _PROGRAM_CACHE[key] = nc
    return nc


def make_core_inputs(x, Wq, Wk, Wv, Wo):
    """Host-side sharding prep. Returns (in_maps, m)."""
    b, s, d = x.shape
    m = b * s
    xT = np.ascontiguousarray(x.reshape(m, d).T)

    def wslice(W, c):
        # lhsT chunks: [p, kc, j] with W[c*HS+j, kc*128+p]
        wt = W[c * HS : (c + 1) * HS, :].T  # [d, HS]
        return np.ascontiguousarray(wt.reshape(KC, 128, HS).transpose(1, 0, 2))

    in_maps = []
    for c in range(NCORES):
        in_maps.append(
            {
                "xT": xT,
                "wq": wslice(Wq, c),
                "wk": wslice(Wk, c),
                "wv": wslice(Wv, c),
                "wo": np.ascontiguousarray(Wo[:, c * HS : (c + 1) * HS].T),
            }
        )
    return in_maps, m


def kernel(x, Wq, Wk, Wv, Wo):
    global LAST_EXEC_NS, LAST_RESULTS
    x = np.asarray(x, dtype=np.float32)
    Wq = np.asarray(Wq, dtype=np.float32)
    Wk = np.asarray(Wk, dtype=np.float32)
    Wv = np.asarray(Wv, dtype=np.float32)
    Wo = np.asarray(Wo, dtype=np.float32)
    b, s, d = x.shape

    from concourse import bass_utils

    nc = build_program(b, s)
    in_maps, m = make_core_inputs(x, Wq, Wk, Wv, Wo)
    res = bass_utils.run_bass_kernel_spmd(
        nc, in_maps, list(range(NCORES)), trace=PROFILE
    )
    LAST_EXEC_NS = res.exec_time_ns
    LAST_RESULTS = res
    out = res.results[0]["out"].astype(np.float64)
    for c in range(1, NCORES):
        out += res.results[c]["out"]
    return out.astype(np.float32).reshape(b, s, d)

